# revision 22
# baseline (speedup 1.0000x reference)
"""Trainium2 Bass kernel for nn_ACRoPEAttention (axial RoPE attention).

Sharding: sequence-parallel. 8 cores = 2 batches x 4 token-chunks of 512.
Each core computes q/k/v (all 16 heads) for its 512 tokens, RoPEs them,
AllGathers k^T and v within its 4-core batch group, runs full attention for
its 512 queries, and projects. Output is token-sharded -> host concat.

Device dataflow is fully "transposed" (channels on partitions):
  qkv^T = Wqkv^T-as-lhsT matmuls over x^T;  RoPE pair-swap via a constant
  S-matrix matmul;  scores^T per head = k^T-as-lhsT @ q^T (row-paired 2
  heads/matmul);  softmax = exp (no max-sub; scores are O(1)) + denominator
  via ones-lhsT col-tiled matmuls;  att@v via v-as-lhsT col-paired matmuls;
  y^T = Wproj^T-as-lhsT @ attn^T.  All matmul operands bf16 (fp32 accum).
"""

import sys

import numpy as np
import ml_dtypes

if "/opt/trn_rl_repo" not in sys.path:
    sys.path.insert(0, "/opt/trn_rl_repo")

BF16 = ml_dtypes.bfloat16

NUM_HEADS = 16
GRID_SIZE = 16
B, N, C = 2, 2048, 1024
HD = C // NUM_HEADS          # 64
NCORES = 8
CHUNK = N // 4               # 512 tokens per core
NPAIR = NUM_HEADS // 2       # 8 head-pair tiles of 128 partitions
KCH = N // 128               # 16 k-chunks of 128 tokens

_CACHE = {}


# ----------------------------------------------------------------- host prep

def _build_tables(T, H, W):
    """Full-token cos/sin tables [N, 64] float64 (tiled-repeat RoPE layout)."""
    n = T * H * W
    ids = np.arange(n)
    d_pos = (ids // (H * W)).astype(np.float64)
    rem = ids % (H * W)
    h_pos = (rem // W).astype(np.float64) * (GRID_SIZE / H)
    w_pos = (rem % W).astype(np.float64) * (GRID_SIZE / W)
    half = 10
    omega = 1.0 / (10000.0 ** (np.arange(half, dtype=np.float64) / half))
    cos_full = np.ones((n, HD), np.float64)
    sin_full = np.zeros((n, HD), np.float64)
    for seg, pos in enumerate([d_pos, h_pos, w_pos]):
        freq = pos[:, None] * omega[None, :]
        cos_full[:, seg * 20:(seg + 1) * 20] = np.tile(np.cos(freq), (1, 2))
        sin_full[:, seg * 20:(seg + 1) * 20] = np.tile(np.sin(freq), (1, 2))
    return cos_full, sin_full


def _build_S128():
    """S such that matmul(out, lhsT=S, rhs=q^T) gives out[2i]=-q[2i+1],
    out[2i+1]=q[2i] for dims<60 of each 64-dim head block (2 blocks)."""
    S = np.zeros((128, 128), np.float32)
    for blk in (0, 64):
        for i in range(30):
            S[blk + 2 * i + 1, blk + 2 * i] = -1.0
            S[blk + 2 * i, blk + 2 * i + 1] = 1.0
    return S


# ------------------------------------------------------------- graph builder

def _build_nc():
    import concourse.bass as bass
    import concourse.mybir as mybir
    import concourse.tile as tile
    from concourse import bacc
    from concourse.bass import broadcast_tensor_aps

    f32 = mybir.dt.float32
    bf16 = mybir.dt.bfloat16
    Exp = mybir.ActivationFunctionType.Exp
    mult = mybir.AluOpType.mult
    add = mybir.AluOpType.add

    nc = bacc.Bacc(None, num_devices=NCORES)

    # parameters (per-core shards / replicated)
    xT = nc.declare_dram_parameter("xT", [C, CHUNK], bf16, isOutput=False)
    wqkvT = nc.declare_dram_parameter("wqkvT", [C, 3 * C], bf16, isOutput=False)
    wprojT = nc.declare_dram_parameter("wprojT", [C, C], bf16, isOutput=False)
    bproj = nc.declare_dram_parameter("bproj", [C, 1], f32, isOutput=False)
    cosq_p = nc.declare_dram_parameter("cosq", [128, CHUNK], f32, isOutput=False)
    sinq_p = nc.declare_dram_parameter("sinq", [128, CHUNK], f32, isOutput=False)
    cosk_p = nc.declare_dram_parameter("cosk", [128, CHUNK], f32, isOutput=False)
    sink_p = nc.declare_dram_parameter("sink", [128, CHUNK], f32, isOutput=False)
    smat_p = nc.declare_dram_parameter("smat", [128, 128], bf16, isOutput=False)
    ones_p = nc.declare_dram_parameter("ones1", [128, 1], bf16, isOutput=False)
    selb_p = nc.declare_dram_parameter("selb", [4, 256], mybir.dt.float32r,
                                       isOutput=False)
    gath_p = nc.declare_dram_parameter("gath", [128, 4], mybir.dt.float32r,
                                       isOutput=False)
    out_p = nc.declare_dram_parameter("out", [C, CHUNK], f32, isOutput=True)

    with tile.TileContext(nc) as tc:
        with (
            tc.tile_pool(name="const", bufs=1) as constp,
            tc.tile_pool(name="dram", bufs=1, space="DRAM") as dramp,
            tc.tile_pool(name="qro", bufs=8) as qrop,
            tc.tile_pool(name="kf", bufs=8) as kfp,
            tc.tile_pool(name="vf", bufs=16) as vfp,
            tc.tile_pool(name="wp", bufs=8) as wpp,
            tc.tile_pool(name="attn", bufs=8) as attnp,
            tc.tile_pool(name="avsb", bufs=4) as avsbp,
            tc.tile_pool(name="dens", bufs=1) as densp,
        ):
            # ---- constants
            cosq = constp.tile([128, CHUNK], f32, tag="cosq")
            sinq = constp.tile([128, CHUNK], f32, tag="sinq")
            cosk = constp.tile([128, CHUNK], f32, tag="cosk")
            sink = constp.tile([128, CHUNK], f32, tag="sink")
            smat = constp.tile([128, 128], bf16, tag="smat")
            ones1 = constp.tile([128, 1], bf16, tag="ones1")
            selb = constp.tile([4, 256], mybir.dt.float32r, tag="selb")
            gath = constp.tile([128, 4], mybir.dt.float32r, tag="gath")
            bias = constp.tile([128, 8], f32, tag="bias")
            nc.sync.dma_start(selb, selb_p[:, :])
            nc.sync.dma_start(gath, gath_p[:, :])
            nc.sync.dma_start(cosq, cosq_p[:, :])
            nc.sync.dma_start(sinq, sinq_p[:, :])
            nc.sync.dma_start(cosk, cosk_p[:, :])
            nc.sync.dma_start(sink, sink_p[:, :])
            nc.sync.dma_start(smat, smat_p[:, :])
            nc.sync.dma_start(ones1, ones_p[:, :])
            # bias [1024,1] -> [128, 8]: col c = b[c*128 : (c+1)*128]
            for c in range(8):
                nc.sync.dma_start(bias[:, c:c + 1],
                                  bproj[c * 128:(c + 1) * 128, :])

            # ---- bounce buffers for AllGather of (k^T pairs, v tiles)
            agin = dramp.tile([16, 128, 512], bf16, tag="agin")
            agout = dramp.tile([4, 16, 128, 512], bf16, tag="agout")

            with (
                tc.tile_pool(name="xw", bufs=1) as xwp,
                tc.tile_pool(name="kvloc", bufs=1) as kvlocp,
                tc.tile_pool(name="rtmp", bufs=3) as rtmpp,
                tc.tile_pool(name="qkpsum", bufs=3, space="PSUM") as qkpsump,
                tc.tile_pool(name="swpsum", bufs=2, space="PSUM") as swpsump,
            ):
                # ---- load x^T and W_qkv^T
                xt = []
                for c in range(8):
                    t = xwp.tile([128, CHUNK], bf16, name=f"xt{c}", tag=f"xt{c}")
                    nc.sync.dma_start(t, xT[c * 128:(c + 1) * 128, :])
                    xt.append(t)
                wq = []
                for c in range(8):
                    t = xwp.tile([128, 3 * C], bf16, name=f"wq{c}", tag=f"wq{c}")
                    nc.sync.dma_start(t, wqkvT[c * 128:(c + 1) * 128, :])
                    wq.append(t)

                def rope_pair(p, which):
                    """Compute roped (q|k)^T pair tile p -> returns SBUF bf16 tile."""
                    off = (0 if which == "q" else C) + p * 128
                    cost = cosq if which == "q" else cosk
                    sint = sinq if which == "q" else sink
                    ps = qkpsump.tile([128, CHUNK], f32, name=f"{which}ps{p}",
                                      tag="qkps")
                    for c in range(8):
                        nc.tensor.matmul(ps, wq[c][:, off:off + 128], xt[c],
                                         start=(c == 0), stop=(c == 7))
                    raw = rtmpp.tile([128, CHUNK], bf16, name=f"{which}raw{p}",
                                     tag="raw")
                    nc.vector.tensor_copy(raw, ps)
                    sw = swpsump.tile([128, CHUNK], f32, name=f"{which}sw{p}",
                                      tag="swp")
                    nc.tensor.matmul(sw, smat, raw, start=True, stop=True)
                    t1 = rtmpp.tile([128, CHUNK], f32, name=f"{which}t1{p}",
                                    tag="t1")
                    nc.vector.tensor_tensor(t1, sw, sint, mult)
                    t2 = rtmpp.tile([128, CHUNK], f32, name=f"{which}t2{p}",
                                    tag="t2")
                    nc.vector.tensor_tensor(t2, raw, cost, mult)
                    if which == "q":
                        ro = qrop.tile([128, CHUNK], bf16, name=f"qro{p}", tag="qro")
                    else:
                        ro = kvlocp.tile([128, CHUNK], bf16, name=f"kro{p}",
                                         tag=f"kro{p}")
                    nc.vector.tensor_tensor(ro, t1, t2, add)
                    return ro

                # ---- k first (feeds AllGather), then v, then q during AG
                for p in range(NPAIR):
                    kro = rope_pair(p, "k")
                    nc.sync.dma_start(agin[p], kro)
                for i in range(4):
                    vloc = kvlocp.tile([128, C], bf16, name=f"vloc{i}",
                                       tag=f"vloc{i}")
                    for oc in range(2):
                        ps = qkpsump.tile([128, 512], f32, name=f"vps{i}{oc}",
                                          tag="qkps")
                        for c in range(8):
                            nc.tensor.matmul(
                                ps,
                                xt[c][:, i * 128:(i + 1) * 128],
                                wq[c][:, 2 * C + oc * 512:2 * C + (oc + 1) * 512],
                                start=(c == 0), stop=(c == 7))
                        nc.vector.tensor_copy(vloc[:, oc * 512:(oc + 1) * 512], ps)
                    nc.sync.dma_start(agin[8 + 2 * i], vloc[:, 0:512])
                    nc.sync.dma_start(agin[8 + 2 * i + 1], vloc[:, 512:1024])

                nc.gpsimd.collective_compute(
                    "AllGather",
                    mybir.AluOpType.bypass,
                    replica_groups=[[0, 1, 2, 3], [4, 5, 6, 7]],
                    ins=[agin.opt()],
                    outs=[agout.opt()],
                )

                qro = [rope_pair(p, "q") for p in range(NPAIR)]

            # ---- prefetch W_proj^T
            wp = []
            for c in range(8):
                t = wpp.tile([128, C], bf16, name=f"wp{c}", tag="wp")
                nc.sync.dma_start(t, wprojT[c * 128:(c + 1) * 128, :])
                wp.append(t)

            # ---- read back gathered k^T / v
            kf = []
            for p in range(NPAIR):
                t = kfp.tile([128, N], bf16, name=f"kf{p}", tag="kf")
                for j in range(4):
                    nc.sync.dma_start(t[:, j * 512:(j + 1) * 512], agout[j, p])
                kf.append(t)
            vf = []
            for i in range(16):
                t = vfp.tile([128, C], bf16, name=f"vf{i}", tag="vf")
                j, ii = divmod(i, 4)
                nc.sync.dma_start(t[:, 0:512], agout[j, 8 + 2 * ii])
                nc.sync.dma_start(t[:, 512:1024], agout[j, 8 + 2 * ii + 1])
                vf.append(t)

            # ---- attention
            recipsb = densp.tile([4, 4 * CHUNK], mybir.dt.float32r, tag="recip")
            avsb = {}
            with (
                tc.tile_pool(name="scps", bufs=2, space="PSUM") as scpsp,
                tc.tile_pool(name="avps", bufs=1, space="PSUM") as avpsp,
                tc.tile_pool(name="dnps", bufs=1, space="PSUM") as dnpsp,
                tc.tile_pool(name="rbps", bufs=2, space="PSUM") as rbpsp,
                tc.tile_pool(name="pt", bufs=3) as ptp,
                tc.tile_pool(name="dnsb", bufs=2) as dnsbp,
            ):
                for pr in range(NPAIR):
                    h0, h1 = 2 * pr, 2 * pr + 1
                    av = avpsp.tile([128, CHUNK], f32, name=f"av{pr}", tag="av")
                    if pr % 2 == 0:
                        dn = dnpsp.tile([128, CHUNK], f32, name=f"dn{pr // 2}",
                                        tag="dn")
                    ro = 64 * (pr % 2)  # denom row base for this pair in quad tile
                    for g in range(KCH):
                        sc = scpsp.tile([128, 1024], f32, name=f"sc{pr}_{g}",
                                        tag="sc")
                        nc.tensor.matmul(sc[:, 0:512],
                                         kf[pr][0:64, g * 128:(g + 1) * 128],
                                         qro[pr][0:64, :], start=True, stop=True)
                        nc.tensor.matmul(sc[:, 512:1024],
                                         kf[pr][64:128, g * 128:(g + 1) * 128],
                                         qro[pr][64:128, :], start=True, stop=True)
                        pt = ptp.tile([128, 1024], bf16, name=f"pt{pr}_{g}",
                                      tag="pt")
                        nc.scalar.activation(pt, sc, Exp)
                        first, last = (g == 0), (g == KCH - 1)
                        nc.tensor.matmul(av[0:64, :],
                                         vf[g][:, h0 * 64:(h0 + 1) * 64],
                                         pt[:, 0:512], start=first, stop=last,
                                         skip_group_check=True)
                        nc.tensor.matmul(av[64:128, :],
                                         vf[g][:, h1 * 64:(h1 + 1) * 64],
                                         pt[:, 512:1024], start=first, stop=last,
                                         skip_group_check=True)
                        nc.tensor.matmul(dn[ro:ro + 1, :], ones1,
                                         pt[:, 0:512], start=first, stop=last,
                                         skip_group_check=True,
                                         tile_position=(0, ro))
                        nc.tensor.matmul(dn[ro + 32:ro + 33, :], ones1,
                                         pt[:, 512:1024], start=first, stop=last,
                                         skip_group_check=True,
                                         tile_position=(0, ro + 32))
                    # unnormalized attn out -> SBUF (frees av bank)
                    t = avsbp.tile([128, CHUNK], bf16, name=f"avsb{pr}", tag="avsb")
                    nc.vector.tensor_copy(t, av)
                    avsb[pr] = t
                    if pr % 2 == 1:
                        q4 = pr // 2  # quad index: heads 4q..4q+3 in dn rows 0/32/64/96
                        qs = slice(q4 * CHUNK, (q4 + 1) * CHUNK)
                        dnsb = dnsbp.tile([128, CHUNK], mybir.dt.float32r,
                                          name=f"dnsb{q4}", tag="dnsb")
                        with nc.allow_low_precision(
                                reason="f32r is bit-identical to f32"):
                            nc.vector.tensor_copy(dnsb, dn)
                            # gather the 4 denom rows {0,32,64,96} -> rows 0..3
                            dng = rbpsp.tile([4, CHUNK], f32, name=f"dng{q4}",
                                             tag="rb")
                            nc.tensor.matmul(dng, gath, dnsb,
                                             start=True, stop=True)
                            nc.vector.reciprocal(recipsb[:, qs], dng)
                        for pq in (pr - 1, pr):
                            rb = rbpsp.tile([128, CHUNK], f32, name=f"rb{pq}",
                                            tag="rb")
                            nc.tensor.matmul(
                                rb, selb[:, (pq % 2) * 128:(pq % 2 + 1) * 128],
                                recipsb[:, qs], start=True, stop=True)
                            at = attnp.tile([128, CHUNK], bf16,
                                            name=f"attn{pq}", tag="attn")
                            nc.vector.tensor_tensor(at, avsb[pq], rb, mult)
                            avsb[pq] = at

            attnT = [avsb[p] for p in range(NPAIR)]

            # ---- output projection + bias
            with (
                tc.tile_pool(name="yps", bufs=2, space="PSUM") as ypsp,
                tc.tile_pool(name="ysb", bufs=2) as ysbp,
            ):
                for oc in range(8):
                    yp = ypsp.tile([128, CHUNK], f32, name=f"yp{oc}", tag="yp")
                    for c in range(8):
                        nc.tensor.matmul(yp, wp[c][:, oc * 128:(oc + 1) * 128],
                                         attnT[c], start=(c == 0), stop=(c == 7))
                    ysb = ysbp.tile([128, CHUNK], f32, name=f"ysb{oc}", tag="ysb")
                    nc.vector.tensor_scalar(ysb, yp, bias[:, oc:oc + 1], None, add)
                    nc.sync.dma_start(out_p[oc * 128:(oc + 1) * 128, :], ysb)

    nc.finalize()
    return nc


# ------------------------------------------------------------------- kernel

def prepare_in_maps(x, W_qkv, W_proj, b_proj, T, H, W):
    T, H, W_ = int(T), int(H), int(W)
    x = np.asarray(x, np.float32)
    W_qkv = np.asarray(W_qkv, np.float32)
    W_proj = np.asarray(W_proj, np.float32)
    b_proj = np.asarray(b_proj, np.float32)
    assert x.shape == (B, N, C) and T * H * W_ == N

    scale = HD ** -0.5
    cos_full, sin_full = _build_tables(T, H, W_)
    wqkvT = np.ascontiguousarray(W_qkv.T).astype(BF16)
    wprojT = np.ascontiguousarray(W_proj.T).astype(BF16)
    smat = _build_S128().astype(BF16)
    ones1 = np.ones((128, 1), BF16)
    bproj2 = b_proj.reshape(C, 1).astype(np.float32)
    # selb[:, 0:128]: even pair of a quad (rows 0,1); [:, 128:256]: odd (rows 2,3)
    selb = np.zeros((4, 256), np.float32)
    selb[0, 0:64] = 1.0
    selb[1, 64:128] = 1.0
    selb[2, 128:192] = 1.0
    selb[3, 192:256] = 1.0
    gath = np.zeros((128, 4), np.float32)
    for r in range(4):
        gath[32 * r, r] = 1.0

    in_maps = []
    for core in range(NCORES):
        b, j = divmod(core, 4)
        r0 = j * CHUNK
        sl = slice(r0, r0 + CHUNK)
        cos_l = cos_full[sl].T
        sin_l = sin_full[sl].T
        cq = np.concatenate([cos_l, cos_l], 0) * scale
        sq = np.concatenate([sin_l, sin_l], 0) * scale
        ck = np.concatenate([cos_l, cos_l], 0)
        sk = np.concatenate([sin_l, sin_l], 0)
        in_maps.append({
            "xT": np.ascontiguousarray(x[b, sl, :].T).astype(BF16),
            "wqkvT": wqkvT,
            "wprojT": wprojT,
            "bproj": bproj2,
            "cosq": np.ascontiguousarray(cq, np.float32),
            "sinq": np.ascontiguousarray(sq, np.float32),
            "cosk": np.ascontiguousarray(ck, np.float32),
            "sink": np.ascontiguousarray(sk, np.float32),
            "smat": smat,
            "ones1": ones1,
            "selb": selb,
            "gath": gath,
        })
    return in_maps


def assemble_output(results):
    y = np.empty((B, N, C), np.float32)
    for core in range(NCORES):
        b, j = divmod(core, 4)
        r0 = j * CHUNK
        y[b, r0:r0 + CHUNK, :] = results[core]["out"].T
    return y


def get_nc():
    if "nc" not in _CACHE:
        _CACHE["nc"] = _build_nc()
    return _CACHE["nc"]


def kernel(x, W_qkv, W_proj, b_proj, T, H, W):
    from concourse.bass_utils import run_bass_kernel_spmd

    nc = get_nc()
    in_maps = prepare_in_maps(x, W_qkv, W_proj, b_proj, T, H, W)
    res = run_bass_kernel_spmd(nc, in_maps, core_ids=list(range(NCORES)))
    return assemble_output(res.results)


if __name__ == "__main__":
    rng = np.random.default_rng(0)
    inp = {
        "x": rng.standard_normal((B, N, C), np.float32),
        "W_qkv": rng.standard_normal((3 * C, C), np.float32) * 0.02,
        "W_proj": rng.standard_normal((C, C), np.float32) * 0.02,
        "b_proj": rng.standard_normal(C, np.float32) * 0.02,
        "T": 8, "H": 16, "W": 16,
    }
    out = kernel(**inp)
    print(out.shape, out.dtype)
